# revision 18
# baseline (speedup 1.0000x reference)
"""Trainium2 Bass kernel for nn_LossSupervisedTags (tag + heatmap MSE loss).

Contract: kernel(**inputs) takes the FULL unsharded inputs (as produced by
setup_inputs) and returns the FULL scalar output.  Internally the batch dim
(B=32) is sharded 4-images-per-core across 8 NeuronCores; each core computes
its local tag / heatmap loss partial sums on device, and the host combines
the 8 partial sums into the final scalar mean.

Host staging: slices per-core shards, transposes dets/heat to [h, p, w] so
every DMA descriptor is an 8.7KB contiguous run (descriptor-bound otherwise),
and gathers the 510 predicted tags per image (index staging for the tag loss).

Per-core device pipeline (per image b, stacks s=0..3):
  DVE   : diff = dets[b,s] - heat[b]          (layout [h=128, (p,w)=2176])
  ACT   : sq   = diff^2  (Square), contiguous into sq_img[:, s*2176:...]
  PE    : for each w (4-way column tiling): psum[32j] += mask[:,w]^T @ sq[..w]
          (contracts h; accumulates over w in PSUM -> sum_{h,w} m*sq per (s,p))
  tag   : gathered pred tags packed [128,64]; (pt-gt)^2*vis summed on DVE.
"""

import sys
import types

import ml_dtypes
import numpy as np

import concourse.bacc as bacc
import concourse.mybir as mybir
from concourse.tile import TileContext
from concourse.bass_utils import run_bass_kernel_spmd

# If BASS_TRACE is set in the environment but this image lacks
# antenv.axon_hooks, run_bass_kernel_spmd would die on import; register a
# no-op hook module so tracing degrades gracefully instead.
try:
    import antenv.axon_hooks  # noqa: F401
except ImportError:
    try:
        import antenv

        _m = types.ModuleType("antenv.axon_hooks")
        _m.get_axon_ntff_profile_hook = lambda: None
        _m.set_axon_ntff_profile_hook = lambda h: None
        sys.modules["antenv.axon_hooks"] = _m
        antenv.axon_hooks = _m
    except ImportError:
        pass

# Problem constants (hardcoded per harness contract)
B, S, C, H, W = 32, 4, 34, 128, 128
N_PARTS, TAG_DIM, M = 17, 1, 30
TAG_W, HM_W = 0.001, 1.0
NCORES = 8
BLOC = B // NCORES            # 4 images per core
FREE = N_PARTS * W            # 2176 free elems per (b, s) tile
SQF = S * N_PARTS             # 68 output columns per image (s, p)
KP = M * N_PARTS              # 510 keypoints per image
KP_COLS = 4                   # ceil(510 / 128) columns per (b, s)
TAG_COLS = BLOC * S * KP_COLS  # 64

_cache = {}


def _build():
    f32 = mybir.dt.float32
    nc = bacc.Bacc(
        "TRN2", target_bir_lowering=False, debug=False, num_devices=NCORES
    )
    # dets/heat pre-transposed on host to [.., H, N_PARTS, W] so the DMA's
    # partition dim (h) has an 8.7KB contiguous run per partition.
    bf16 = mybir.dt.bfloat16
    dets = nc.dram_tensor(
        "dets", [BLOC, S, H, N_PARTS, W], bf16, kind="ExternalInput"
    )
    heat = nc.dram_tensor("heat", [BLOC, H, N_PARTS, W], bf16, kind="ExternalInput")
    maskw = nc.dram_tensor("maskw", [H, BLOC * W], f32, kind="ExternalInput")
    tagin = nc.dram_tensor("tagin", [128, 3 * TAG_COLS], f32, kind="ExternalInput")
    out_det = nc.dram_tensor("out_det", [4, BLOC * SQF], f32, kind="ExternalOutput")
    out_tag = nc.dram_tensor("out_tag", [128, 2], f32, kind="ExternalOutput")

    with TileContext(nc) as tc:
        with (
            tc.tile_pool(name="const", bufs=1) as cpool,
            tc.tile_pool(name="heatp", bufs=2) as hpool,
            tc.tile_pool(name="detp", bufs=7) as dpool,
            tc.tile_pool(name="diffp", bufs=4) as fpool,
            tc.tile_pool(name="sqp", bufs=3) as qpool,
            tc.tile_pool(name="psum", bufs=2, space="PSUM") as ppool,
        ):
            # Issue the first image's heat + det DMAs before anything else so
            # the compute pipeline starts as early as possible (outstanding
            # DMAs round-robin at packet granularity, so whatever is queued
            # first-and-alone completes first).
            heat_tiles = {}
            det_tiles = {}
            heat_tiles[0] = hpool.tile([128, FREE], bf16, name="heat_t", tag="heat_t")
            nc.sync.dma_start(
                out=heat_tiles[0][:], in_=heat[0].rearrange("h p w -> h (p w)")
            )
            det_tiles[(0, 0)] = dpool.tile([128, FREE], bf16, name="det_t", tag="det_t")
            nc.sync.dma_start(
                out=det_tiles[(0, 0)][:], in_=dets[0, 0].rearrange("h p w -> h (p w)")
            )

            mask_t = cpool.tile([128, BLOC * W], f32)
            acc_det = cpool.tile([128, BLOC * SQF], f32)
            det33_acc = cpool.tile([128, 1], f32)


            # mask arrives well before the first matmul needs it
            nc.sync.dma_start(out=mask_t[:], in_=maskw[:])

            # ---- heatmap (det) loss ----
            for b in range(BLOC):
                if b in heat_tiles:
                    heat_t = heat_tiles[b]
                else:
                    heat_t = hpool.tile([128, FREE], bf16, name="heat_t", tag="heat_t")
                    nc.sync.dma_start(
                        out=heat_t[:], in_=heat[b].rearrange("h p w -> h (p w)")
                    )
                sq_t = qpool.tile([128, S * FREE], f32)
                sq_r = sq_t[:].rearrange("q (s p w) -> q s p w", s=S, p=N_PARTS)
                for s in range(S):
                    if (b, s) in det_tiles:
                        det_t = det_tiles[(b, s)]
                    else:
                        det_t = dpool.tile([128, FREE], bf16, name="det_t", tag="det_t")
                        nc.sync.dma_start(
                            out=det_t[:], in_=dets[b, s].rearrange("h p w -> h (p w)")
                        )
                    diff_t = fpool.tile([128, FREE], bf16, name="diff_t", tag="diff_t")
                    nc.vector.tensor_sub(diff_t[:], det_t[:], heat_t[:])
                    # ACT is the pipeline pacer; offload one square per image
                    # to the cheaper DVE bf16 path (2x mode) to rebalance.
                    if s == 1:
                        nc.vector.tensor_mul(
                            sq_t[:, s * FREE : (s + 1) * FREE], diff_t[:], diff_t[:]
                        )
                    else:
                        nc.scalar.activation(
                            sq_t[:, s * FREE : (s + 1) * FREE],
                            diff_t[:],
                            mybir.ActivationFunctionType.Square,
                        )
                # 4-way column-tiled mask-weighted reduction on PE:
                # group j = w % 4 accumulates into psum partition 32j.
                # For the last image, PE covers s=0..2 only and the (3,3)
                # stack goes to the otherwise-idle DVE, shortening the tail.
                ns = 3 if b == BLOC - 1 else S
                nf = ns * N_PARTS
                psum_t = ppool.tile([128, SQF], f32)
                for w in range(W):
                    j = w % 4
                    nc.tensor.matmul(
                        psum_t[32 * j : 32 * j + 1, 0:nf],
                        lhsT=mask_t[:, b * W + w : b * W + w + 1],
                        rhs=sq_r[:, 0:ns, :, w],
                        start=(w < 4),
                        stop=(w >= W - 4),
                        tile_position=(0, 32 * j),
                    )
                for j in range(4):
                    nc.scalar.copy(
                        acc_det[32 * j : 32 * j + 1, b * SQF : b * SQF + nf],
                        psum_t[32 * j : 32 * j + 1, 0:nf],
                    )
                if b == BLOC - 1:
                    mb = (
                        mask_t[:, b * W : (b + 1) * W]
                        .unsqueeze(1)
                        .broadcast_to([128, N_PARTS, W])
                    )
                    sqm_t = fpool.tile([128, FREE], f32)
                    nc.vector.tensor_mul(
                        sqm_t[:].rearrange("q (p w) -> q p w", p=N_PARTS),
                        sq_r[:, S - 1],
                        mb,
                    )
                    nc.vector.reduce_sum(
                        det33_acc[:], sqm_t[:], axis=mybir.AxisListType.X
                    )

            # ---- tag loss (tiny) ----
            tag_t = cpool.tile([128, 3 * TAG_COLS], f32)
            nc.sync.dma_start(out=tag_t[:], in_=tagin[:])
            ptg_t = tag_t[:, 0:TAG_COLS]
            gtv_t = tag_t[:, TAG_COLS : 2 * TAG_COLS]
            vis_t = tag_t[:, 2 * TAG_COLS : 3 * TAG_COLS]
            e_t = cpool.tile([128, TAG_COLS], f32)
            ev_t = cpool.tile([128, TAG_COLS], f32)
            scr_t = cpool.tile([128, TAG_COLS], f32)
            tag_acc = cpool.tile([128, 2], f32)
            nc.vector.tensor_sub(e_t[:], ptg_t, gtv_t)
            nc.vector.tensor_mul(ev_t[:], e_t[:], vis_t)
            nc.vector.tensor_mul(scr_t[:], e_t[:], ev_t[:])
            nc.vector.reduce_sum(tag_acc[:, 0:1], scr_t[:], axis=mybir.AxisListType.X)
            nc.scalar.copy(tag_acc[:, 1:2], det33_acc[:])
            nc.sync.dma_start(out=out_tag[:], in_=tag_acc[:])

            nc.sync.dma_start(out=out_det[:], in_=acc_det[0:128:32, :])
    nc.compile()
    return nc


def _pack(vals):
    """vals: (BLOC, S, KP) float32 -> [128, TAG_COLS] with col = b*16+s*4+j,
    partition k holding element j*128+k of the zero-padded 512 vector."""
    padded = np.zeros((BLOC, S, KP_COLS * 128), np.float32)
    padded[..., :KP] = vals
    return (
        padded.reshape(BLOC, S, KP_COLS, 128)
        .transpose(3, 0, 1, 2)
        .reshape(128, TAG_COLS)
    )


def kernel(preds, masks, keypoints_idx, keypoints_vis, gt_tags, heatmaps):
    preds = np.asarray(preds, dtype=np.float32)
    masks = np.asarray(masks, dtype=np.float32)
    keypoints_idx = np.asarray(keypoints_idx)
    keypoints_vis = np.asarray(keypoints_vis, dtype=np.float32)
    gt_tags = np.asarray(gt_tags, dtype=np.float32)
    heatmaps = np.asarray(heatmaps, dtype=np.float32)

    if "nc" not in _cache:
        _cache["nc"] = _build()
    nc = _cache["nc"]

    # Host-side input staging: gather predicted tags at keypoint locations
    # (index-based staging; all loss arithmetic runs on device).
    tags = preds[:, :, N_PARTS:].reshape(B, S, N_PARTS * H * W)
    flat_idx = keypoints_idx.reshape(B, 1, KP).astype(np.int64)
    pt = np.take_along_axis(tags, np.broadcast_to(flat_idx, (B, S, KP)), axis=2)
    gt = gt_tags.reshape(B, KP)
    vi = keypoints_vis.reshape(B, KP)

    in_maps = []
    for c in range(NCORES):
        b0 = c * BLOC
        sl = slice(b0, b0 + BLOC)
        tag_in = np.concatenate(
            [
                _pack(pt[sl]),
                _pack(np.broadcast_to(gt[sl][:, None, :], (BLOC, S, KP))),
                _pack(np.broadcast_to(vi[sl][:, None, :], (BLOC, S, KP))),
            ],
            axis=1,
        )
        in_maps.append(
            {
                # [BLOC, S, 17, H, W] -> [BLOC, S, H, 17, W]
                "dets": np.ascontiguousarray(
                    preds[sl, :, :N_PARTS].transpose(0, 1, 3, 2, 4)
                ).astype(ml_dtypes.bfloat16),
                # [BLOC, 17, H, W] -> [BLOC, H, 17, W]
                "heat": np.ascontiguousarray(
                    heatmaps[sl].transpose(0, 2, 1, 3)
                ).astype(ml_dtypes.bfloat16),
                # [BLOC, H, W] -> [H, BLOC*W]
                "maskw": np.ascontiguousarray(
                    masks[sl].transpose(1, 0, 2).reshape(H, BLOC * W)
                ),
                "tagin": np.ascontiguousarray(tag_in),
            }
        )

    res = run_bass_kernel_spmd(nc, in_maps, list(range(NCORES)))
    _cache["last_results"] = res

    det_total = 0.0
    tag_total = 0.0
    ncols = (BLOC - 1) * SQF + 3 * N_PARTS  # last image: s=0..2 on PE
    for r in res.results:
        det_total += float(r["out_det"][:, :ncols].sum(dtype=np.float64))
        det_total += float(r["out_tag"][:, 1].sum(dtype=np.float64))
        tag_total += float(r["out_tag"][:, 0].sum(dtype=np.float64))

    det_mean = det_total / (B * S * N_PARTS * H * W)
    tag_mean = tag_total / (B * S)
    return np.float32(TAG_W * tag_mean + HM_W * det_mean)


# revision 19
# speedup vs baseline: 1.1124x; 1.1124x over previous
"""Trainium2 Bass kernel for nn_LossSupervisedTags (tag + heatmap MSE loss).

Contract: kernel(**inputs) takes the FULL unsharded inputs (as produced by
setup_inputs) and returns the FULL scalar output.  Internally the batch dim
(B=32) is sharded 4-images-per-core across 8 NeuronCores; each core computes
its local tag / heatmap loss partial sums on device, and the host combines
the 8 partial sums into the final scalar mean.

Host staging: slices per-core shards, transposes dets/heat to [h, p, w] so
every DMA descriptor is an 8.7KB contiguous run (descriptor-bound otherwise),
and gathers the 510 predicted tags per image (index staging for the tag loss).

Per-core device pipeline (per image b, stacks s=0..3):
  DVE   : diff = dets[b,s] - heat[b]          (layout [h=128, (p,w)=2176])
  ACT   : sq   = diff^2  (Square), contiguous into sq_img[:, s*2176:...]
  PE    : for each w (4-way column tiling): psum[32j] += mask[:,w]^T @ sq[..w]
          (contracts h; accumulates over w in PSUM -> sum_{h,w} m*sq per (s,p))
  tag   : gathered pred tags packed [128,64]; (pt-gt)^2*vis summed on DVE.
"""

import sys
import types

import ml_dtypes
import numpy as np

import concourse.bacc as bacc
import concourse.mybir as mybir
from concourse.tile import TileContext
from concourse.bass_utils import run_bass_kernel_spmd

# If BASS_TRACE is set in the environment but this image lacks
# antenv.axon_hooks, run_bass_kernel_spmd would die on import; register a
# no-op hook module so tracing degrades gracefully instead.
try:
    import antenv.axon_hooks  # noqa: F401
except ImportError:
    try:
        import antenv

        _m = types.ModuleType("antenv.axon_hooks")
        _m.get_axon_ntff_profile_hook = lambda: None
        _m.set_axon_ntff_profile_hook = lambda h: None
        sys.modules["antenv.axon_hooks"] = _m
        antenv.axon_hooks = _m
    except ImportError:
        pass

# Problem constants (hardcoded per harness contract)
B, S, C, H, W = 32, 4, 34, 128, 128
N_PARTS, TAG_DIM, M = 17, 1, 30
TAG_W, HM_W = 0.001, 1.0
NCORES = 8
BLOC = B // NCORES            # 4 images per core
FREE = N_PARTS * W            # 2176 free elems per (b, s) tile
SQF = S * N_PARTS             # 68 output columns per image (s, p)
KP = M * N_PARTS              # 510 keypoints per image
KP_COLS = 4                   # ceil(510 / 128) columns per (b, s)
TAG_COLS = BLOC * S * KP_COLS  # 64

_cache = {}


def _build():
    f32 = mybir.dt.float32
    nc = bacc.Bacc(
        "TRN2", target_bir_lowering=False, debug=False, num_devices=NCORES
    )
    # dets/heat pre-transposed on host to [.., H, N_PARTS, W] so the DMA's
    # partition dim (h) has an 8.7KB contiguous run per partition.
    bf16 = mybir.dt.bfloat16
    dets = nc.dram_tensor(
        "dets", [BLOC, S, H, N_PARTS, W], bf16, kind="ExternalInput"
    )
    heat = nc.dram_tensor("heat", [BLOC, H, N_PARTS, W], bf16, kind="ExternalInput")
    maskw = nc.dram_tensor("maskw", [H, BLOC * W], f32, kind="ExternalInput")
    tagin = nc.dram_tensor("tagin", [128, 3 * TAG_COLS], f32, kind="ExternalInput")
    out_det = nc.dram_tensor("out_det", [4, BLOC * SQF], f32, kind="ExternalOutput")
    out_tag = nc.dram_tensor("out_tag", [128, 2], f32, kind="ExternalOutput")

    with TileContext(nc) as tc:
        with (
            tc.tile_pool(name="const", bufs=1) as cpool,
            tc.tile_pool(name="heatp", bufs=2) as hpool,
            tc.tile_pool(name="detp", bufs=7) as dpool,
            tc.tile_pool(name="diffp", bufs=2) as fpool,
            tc.tile_pool(name="sqp", bufs=2) as qpool,
            tc.tile_pool(name="psum", bufs=2, space="PSUM") as ppool,
        ):
            # Issue the first image's heat + det DMAs before anything else so
            # the compute pipeline starts as early as possible (outstanding
            # DMAs round-robin at packet granularity, so whatever is queued
            # first-and-alone completes first).
            heat_tiles = {}
            det_tiles = {}
            heat_tiles[0] = hpool.tile([128, FREE], bf16, name="heat_t", tag="heat_t")
            nc.sync.dma_start(
                out=heat_tiles[0][:], in_=heat[0].rearrange("h p w -> h (p w)")
            )
            det_tiles[(0, 0)] = dpool.tile([128, FREE], bf16, name="det_t", tag="det_t")
            nc.sync.dma_start(
                out=det_tiles[(0, 0)][:], in_=dets[0, 0].rearrange("h p w -> h (p w)")
            )

            mask_t = cpool.tile([128, BLOC * W], f32)
            acc_det = cpool.tile([128, BLOC * SQF], f32)
            det33_acc = cpool.tile([128, 1], f32)


            # mask arrives well before the first matmul needs it
            nc.sync.dma_start(out=mask_t[:], in_=maskw[:])

            # ---- heatmap (det) loss ----
            for b in range(BLOC):
                if b in heat_tiles:
                    heat_t = heat_tiles[b]
                else:
                    heat_t = hpool.tile([128, FREE], bf16, name="heat_t", tag="heat_t")
                    nc.sync.dma_start(
                        out=heat_t[:], in_=heat[b].rearrange("h p w -> h (p w)")
                    )
                sq_t = qpool.tile([128, S * FREE], f32)
                sq_r = sq_t[:].rearrange("q (s p w) -> q s p w", s=S, p=N_PARTS)
                for s in range(S):
                    if (b, s) in det_tiles:
                        det_t = det_tiles[(b, s)]
                    else:
                        det_t = dpool.tile([128, FREE], bf16, name="det_t", tag="det_t")
                        nc.sync.dma_start(
                            out=det_t[:], in_=dets[b, s].rearrange("h p w -> h (p w)")
                        )
                    diff_t = fpool.tile([128, FREE], bf16, name="diff_t", tag="diff_t")
                    nc.vector.tensor_sub(diff_t[:], det_t[:], heat_t[:])
                    # ACT is the pipeline pacer; offload one square per image
                    # to the cheaper DVE bf16 path (2x mode) to rebalance.
                    if s == 1:
                        nc.vector.tensor_mul(
                            sq_t[:, s * FREE : (s + 1) * FREE], diff_t[:], diff_t[:]
                        )
                    else:
                        nc.scalar.activation(
                            sq_t[:, s * FREE : (s + 1) * FREE],
                            diff_t[:],
                            mybir.ActivationFunctionType.Square,
                        )
                # 4-way column-tiled mask-weighted reduction on PE:
                # group j = w % 4 accumulates into psum partition 32j.
                # For the last image, PE covers s=0..2 only and the (3,3)
                # stack goes to the otherwise-idle DVE, shortening the tail.
                ns = 3 if b == BLOC - 1 else S
                nf = ns * N_PARTS
                psum_t = ppool.tile([128, SQF], f32)
                for w in range(W):
                    j = w % 4
                    nc.tensor.matmul(
                        psum_t[32 * j : 32 * j + 1, 0:nf],
                        lhsT=mask_t[:, b * W + w : b * W + w + 1],
                        rhs=sq_r[:, 0:ns, :, w],
                        start=(w < 4),
                        stop=(w >= W - 4),
                        tile_position=(0, 32 * j),
                    )
                for j in range(4):
                    nc.scalar.copy(
                        acc_det[32 * j : 32 * j + 1, b * SQF : b * SQF + nf],
                        psum_t[32 * j : 32 * j + 1, 0:nf],
                    )
                if b == BLOC - 1:
                    mb = (
                        mask_t[:, b * W : (b + 1) * W]
                        .unsqueeze(1)
                        .broadcast_to([128, N_PARTS, W])
                    )
                    sqm_t = fpool.tile([128, FREE], f32)
                    nc.vector.tensor_mul(
                        sqm_t[:].rearrange("q (p w) -> q p w", p=N_PARTS),
                        sq_r[:, S - 1],
                        mb,
                    )
                    nc.vector.reduce_sum(
                        det33_acc[:], sqm_t[:], axis=mybir.AxisListType.X
                    )

            # ---- tag loss (tiny) ----
            tag_t = cpool.tile([128, 3 * TAG_COLS], f32)
            nc.sync.dma_start(out=tag_t[:], in_=tagin[:])
            ptg_t = tag_t[:, 0:TAG_COLS]
            gtv_t = tag_t[:, TAG_COLS : 2 * TAG_COLS]
            vis_t = tag_t[:, 2 * TAG_COLS : 3 * TAG_COLS]
            e_t = cpool.tile([128, TAG_COLS], f32)
            ev_t = cpool.tile([128, TAG_COLS], f32)
            scr_t = cpool.tile([128, TAG_COLS], f32)
            tag_acc = cpool.tile([128, 2], f32)
            nc.vector.tensor_sub(e_t[:], ptg_t, gtv_t)
            nc.vector.tensor_mul(ev_t[:], e_t[:], vis_t)
            nc.vector.tensor_mul(scr_t[:], e_t[:], ev_t[:])
            nc.vector.reduce_sum(tag_acc[:, 0:1], scr_t[:], axis=mybir.AxisListType.X)
            nc.scalar.copy(tag_acc[:, 1:2], det33_acc[:])
            nc.sync.dma_start(out=out_tag[:], in_=tag_acc[:])

            nc.sync.dma_start(out=out_det[:], in_=acc_det[0:128:32, :])
    nc.compile()
    return nc


def _pack(vals):
    """vals: (BLOC, S, KP) float32 -> [128, TAG_COLS] with col = b*16+s*4+j,
    partition k holding element j*128+k of the zero-padded 512 vector."""
    padded = np.zeros((BLOC, S, KP_COLS * 128), np.float32)
    padded[..., :KP] = vals
    return (
        padded.reshape(BLOC, S, KP_COLS, 128)
        .transpose(3, 0, 1, 2)
        .reshape(128, TAG_COLS)
    )


def kernel(preds, masks, keypoints_idx, keypoints_vis, gt_tags, heatmaps):
    preds = np.asarray(preds, dtype=np.float32)
    masks = np.asarray(masks, dtype=np.float32)
    keypoints_idx = np.asarray(keypoints_idx)
    keypoints_vis = np.asarray(keypoints_vis, dtype=np.float32)
    gt_tags = np.asarray(gt_tags, dtype=np.float32)
    heatmaps = np.asarray(heatmaps, dtype=np.float32)

    if "nc" not in _cache:
        _cache["nc"] = _build()
    nc = _cache["nc"]

    # Host-side input staging: gather predicted tags at keypoint locations
    # (index-based staging; all loss arithmetic runs on device).
    tags = preds[:, :, N_PARTS:].reshape(B, S, N_PARTS * H * W)
    flat_idx = keypoints_idx.reshape(B, 1, KP).astype(np.int64)
    pt = np.take_along_axis(tags, np.broadcast_to(flat_idx, (B, S, KP)), axis=2)
    gt = gt_tags.reshape(B, KP)
    vi = keypoints_vis.reshape(B, KP)

    in_maps = []
    for c in range(NCORES):
        b0 = c * BLOC
        sl = slice(b0, b0 + BLOC)
        tag_in = np.concatenate(
            [
                _pack(pt[sl]),
                _pack(np.broadcast_to(gt[sl][:, None, :], (BLOC, S, KP))),
                _pack(np.broadcast_to(vi[sl][:, None, :], (BLOC, S, KP))),
            ],
            axis=1,
        )
        in_maps.append(
            {
                # [BLOC, S, 17, H, W] -> [BLOC, S, H, 17, W]
                "dets": np.ascontiguousarray(
                    preds[sl, :, :N_PARTS].transpose(0, 1, 3, 2, 4)
                ).astype(ml_dtypes.bfloat16),
                # [BLOC, 17, H, W] -> [BLOC, H, 17, W]
                "heat": np.ascontiguousarray(
                    heatmaps[sl].transpose(0, 2, 1, 3)
                ).astype(ml_dtypes.bfloat16),
                # [BLOC, H, W] -> [H, BLOC*W]
                "maskw": np.ascontiguousarray(
                    masks[sl].transpose(1, 0, 2).reshape(H, BLOC * W)
                ),
                "tagin": np.ascontiguousarray(tag_in),
            }
        )

    res = run_bass_kernel_spmd(nc, in_maps, list(range(NCORES)))
    _cache["last_results"] = res

    det_total = 0.0
    tag_total = 0.0
    ncols = (BLOC - 1) * SQF + 3 * N_PARTS  # last image: s=0..2 on PE
    for r in res.results:
        det_total += float(r["out_det"][:, :ncols].sum(dtype=np.float64))
        det_total += float(r["out_tag"][:, 1].sum(dtype=np.float64))
        tag_total += float(r["out_tag"][:, 0].sum(dtype=np.float64))

    det_mean = det_total / (B * S * N_PARTS * H * W)
    tag_mean = tag_total / (B * S)
    return np.float32(TAG_W * tag_mean + HM_W * det_mean)
